# revision 29
# baseline (speedup 1.0000x reference)
"""Trainium2 Bass kernel for nn_MetaMultiLinear.

Math (per head h, sample b):
    w[b, k]   = sum_c cond[b, c] * CW[k, c] + cb[k]        k = o*17 + i  (544)
    out[b, o] = sum_i x1[b, i] * w[b, o*17+i]              x1 = [input, 1] (17)

Sharding: head h -> NeuronCore h (8 heads, 8 cores), full B=32768 per core.

Per-core device algorithm (tiles of 128 samples, processed in pairs, with
GRP pairs sharing one input DMA / one output DMA):
  1. DMA [cond | x1pad] for 2*GRP tiles into one SBUF block.
  2. PE transpose (one matmul per pair) -> cond^T at partitions 0-31 / 64-95
     and x1^T at partitions 32-48 / 96-112; ScalarE copies PSUM->SBUF.
  3. Step A (PE, float32r fast mode): W = cond @ CW^T via K=32 matmuls,
     lhsT = cond^T slice (row-tiled at partition base 0 / 64), rhs =
     replicated CW^T. W lands in PSUM (two banks, 272 fp32 each).
  4. Bias (PE): out_psum = x1 @ BiasM^T via K=17 matmul with lhsT = x1^T
     slice, rhs = replicated BiasM^T (start=True of the accumulation group).
  5. Step B multiply (DVE, single 1x pass): tmp = W (*) broadcast(x1),
     reading W straight from PSUM, writing tmp to SBUF.
  6. Step B reduce (PE, float32r): identity-stationary matmul streams tmp;
     the PSUM output AP is a broadcast view (17 streamed columns map onto
     one output element), so the 17 per-o products accumulate in PSUM via
     has_written.
  7. ScalarE copies out_psum -> SBUF (DMA cannot read PSUM), one DMA per
     group to HBM.
"""

import sys

import numpy as np

if "/opt/trn_rl_repo" not in sys.path:
    sys.path.insert(0, "/opt/trn_rl_repo")

N_HEADS, IN_F, COND_IN, OUT_F = 8, 16, 32, 32
B = 32768
INP1 = IN_F + 1  # 17
KW = OUT_F * INP1  # 544
P = 128
GRP = 8  # pairs per DMA group (2*GRP tiles of 128 samples)

_cached_nc = None

# "overlap": PE grouped reduce via overlapping PSUM out-AP (2 matmuls/tile)
# "mm17":    PE grouped reduce via 17 accumulated strided matmuls
REDUCE_MODE = "overlap"
# use float32r (single-pass fast fp32, ~1 cycle/row at N>=256) for the
# heavy matmuls; plain fp32 costs 4 cycles/row
USE_F32R = True


def _build_nc(b_total=B, grp=None, reps=1):
    import concourse.bass as bass  # noqa: F401
    import concourse.mybir as mybir
    import concourse.tile as tile
    from concourse import bacc
    from contextlib import ExitStack

    f32 = mybir.dt.float32
    # float32r: single-pass fast-fp32 PE mode (1 cycle/row at N>=256 vs 4
    # for exact fp32). The BIR verifier requires f32r matmul operands to be
    # *produced* as f32r (rounded at the producer), so the tiles feeding
    # the heavy matmuls are typed f32r end-to-end.
    fr = mybir.dt.float32r if USE_F32R else f32
    nc = bacc.Bacc()
    pairs = b_total // (2 * P)
    if grp is None:
        grp = GRP
    while pairs % grp:
        grp //= 2
    groups = pairs // grp

    # cx: per sample [cond (32) | 1.0 | input (16) | 1.0 | zeros (14)]
    cx_t = nc.dram_tensor("cx", [b_total, 64], f32, kind="ExternalInput")
    # cwrep rows 0-31 / 64-95: CW^T; row 32 / 96: cond_bias (the K=33
    # contraction row pairing with cx's ones column folds the bias into W)
    cwrep_t = nc.dram_tensor("cwrep", [P, KW], fr, kind="ExternalInput")
    ident_t = nc.dram_tensor("ident", [P, P], fr, kind="ExternalInput")
    out_t = nc.dram_tensor("out", [b_total, OUT_F], f32, kind="ExternalOutput")

    with tile.TileContext(nc) as tc, ExitStack() as ctx:
        consts = ctx.enter_context(tc.tile_pool(name="consts", bufs=1))
        ptrin = ctx.enter_context(tc.tile_pool(name="ptrin", bufs=3))
        ptrs = ctx.enter_context(tc.tile_pool(name="ptrs", bufs=4))
        ptmp = ctx.enter_context(tc.tile_pool(name="ptmp", bufs=4))
        pouts = ctx.enter_context(tc.tile_pool(name="pouts", bufs=3))
        pps_tr = ctx.enter_context(tc.tile_pool(name="pps_tr", bufs=2, space="PSUM"))
        pps_w = ctx.enter_context(tc.tile_pool(name="pps_w", bufs=2, space="PSUM"))
        pps_o = ctx.enter_context(tc.tile_pool(name="pps_o", bufs=2, space="PSUM"))

        cw = consts.tile([P, KW], fr)
        nc.sync.dma_start(out=cw, in_=cwrep_t[:])
        idn = consts.tile([P, P], fr)
        nc.sync.dma_start(out=idn, in_=ident_t[:])
        idn32 = idn.bitcast(f32)  # transpose identity must match f32 data
        zz = consts.tile([1, P], f32)
        nc.vector.memset(zz, 0.0)

        # Software pipeline: pair p's reduce phase is emitted after pair
        # p+1's produce phase so the PE (strict FIFO) has W-matmul work to
        # do while the DVE multiply for pair p runs.
        pending = []  # (po, tmps, outs_g, outs_col, flush_group_dma)

        def emit_reduce(item):
            po, tmps, o_g, col, dma = item
            for t in (0, 1):
                tmp = tmps[t]
                if REDUCE_MODE == "overlap":
                    # overlapping PSUM out AP (broadcast view); 17 streamed
                    # passes of 16 o-columns land on the same PSUM addresses
                    # and accumulate via has_written. i-outer/o-inner order
                    # keeps the dst innermost dim step-1/even/8B-aligned as
                    # the fp32r paired PSUM write path requires.
                    for half in (0, 1):
                        rhs = tmp[:, half, :, :].rearrange("p o i -> p i o")
                        oslice = po[:, t, half * 16 : (half + 1) * 16]
                        ov = oslice.unsqueeze(1).broadcast_to([P, INP1, 16])
                        nc.tensor.matmul(
                            ov,
                            idn[:],
                            rhs,
                            start=False,
                            stop=(half == 1),
                            skip_group_check=True,
                        )
                else:
                    # sim-safe: 17 accumulated matmuls, each streaming the
                    # stride-17 slice for one i (N=32 columns).
                    for i in range(INP1):
                        nc.tensor.matmul(
                            po[:, t, :],
                            idn[:],
                            tmp[:, :, :, i],
                            start=False,
                            stop=(i == INP1 - 1),
                            skip_group_check=True,
                        )
                # PSUM -> SBUF (DMA cannot read PSUM)
                nc.scalar.copy(out=o_g[:, col + t, :], in_=po[:, t, :])
            if dma is not None:
                dma()

        for gi_rep in range(groups * reps):
            gi = gi_rep % groups
            gb0 = gi * grp * 2 * P
            trin_g = ptrin.tile([P, 2 * grp, 64], f32)
            nc.sync.dma_start(
                out=trin_g[:],
                in_=cx_t[gb0 : gb0 + 2 * grp * P, :].rearrange(
                    "(t p) c -> p t c", t=2 * grp
                ),
            )
            outs_g = pouts.tile([P, 2 * grp, OUT_F], f32)

            def make_group_dma(o_g=outs_g, base=gb0):
                def dma():
                    nc.sync.dma_start(
                        out=out_t[base : base + 2 * grp * P, :].rearrange(
                            "(t p) o -> p t o", t=2 * grp
                        ),
                        in_=o_g[:],
                    )

                return dma

            for pr in range(grp):
                trin = trin_g[:, 2 * pr : 2 * pr + 2, :].rearrange("p t c -> p (t c)")
                trps = pps_tr.tile([P, P], f32)
                # Fence: a 1x1 normal matmul that carries the semaphore waits
                # (trin DMA, idn DMA, trps slot release). Transpose-mode
                # matmuls only support a single sync-wait in codegen.
                nc.tensor.matmul(
                    trps[0:1, 0:1],
                    trin[:, 0:1],
                    idn32[:, 0:1],
                    start=True,
                    stop=True,
                    skip_group_check=True,
                )
                nc.tensor.transpose(trps[:], trin[:], idn32[:])
                trs = ptrs.tile([P, P], fr)
                nc.scalar.copy(out=trs[:], in_=trps[:])

                po = pps_o.tile([P, 2, OUT_F], f32)
                # Open po's accumulation group (whole bank) with one zero
                # matmul; all reduce matmuls then accumulate (start=False).
                # Two start=True groups cannot share a PSUM bank: start=True
                # marks the entire 2KB bank pending-zero.
                nc.tensor.matmul(
                    po[:, :, :],
                    zz[0:1, 0:P],
                    zz[0:1, 0 : 2 * OUT_F],
                    start=True,
                    stop=False,
                    skip_group_check=True,
                    tile_position=(0, 0),
                )
                tmps = []
                for t in (0, 1):
                    g = t * 64
                    # --- step A (K=33 incl. bias row): W chunks [0:272],
                    #     [272:544]
                    w = pps_w.tile([P, 2, 512], f32)
                    nc.tensor.matmul(
                        w[:, 0, 0:272],
                        trs[g : g + 33, :],
                        cw[g : g + 33, 0:272],
                        start=True,
                        stop=True,
                        tile_position=(g, 0),
                    )
                    nc.tensor.matmul(
                        w[:, 1, 0:272],
                        trs[g : g + 33, :],
                        cw[g : g + 33, 272:544],
                        start=True,
                        stop=True,
                        tile_position=(g, 0),
                    )
                    # --- step B multiply on DVE: tmp = W * broadcast(x1)
                    tmp = ptmp.tile([P, 2, 16, INP1], fr)
                    wv = w[:, :, 0:272].rearrange(
                        "p a (b c) -> p a b c", b=16, c=INP1
                    )
                    x1v = (
                        trin[:, g + 33 : g + 50]
                        .unsqueeze(1)
                        .unsqueeze(1)
                        .broadcast_to([P, 2, 16, INP1])
                    )
                    nc.vector.tensor_mul(tmp[:], wv, x1v)
                    tmps.append(tmp)

                is_last_of_group = pr == grp - 1
                pending.append(
                    (
                        po,
                        tmps,
                        outs_g,
                        2 * pr,
                        make_group_dma() if is_last_of_group else None,
                    )
                )
                if len(pending) > 1:
                    emit_reduce(pending.pop(0))
        while pending:
            emit_reduce(pending.pop(0))

    nc.compile()
    return nc


def _get_nc():
    global _cached_nc
    if _cached_nc is None:
        _cached_nc = _build_nc()
    return _cached_nc


def _make_in_maps(input, cond, cond_weight, cond_bias):
    ident = np.eye(P, dtype=np.float32)
    in_maps = []
    n_heads, b_total = input.shape[0], input.shape[1]
    for h in range(n_heads):
        cx = np.zeros((b_total, 64), np.float32)
        cx[:, :COND_IN] = cond[h]
        cx[:, COND_IN] = 1.0
        cx[:, COND_IN + 1 : COND_IN + 1 + IN_F] = input[h]
        cx[:, COND_IN + 1 + IN_F] = 1.0
        cwT = np.ascontiguousarray(cond_weight[h].T)  # (32, 544)
        cwrep = np.zeros((P, KW), np.float32)
        cwrep[0:32] = cwT
        cwrep[32] = cond_bias[h]
        cwrep[64:96] = cwT
        cwrep[96] = cond_bias[h]
        in_maps.append({"cx": cx, "cwrep": cwrep, "ident": ident})
    return in_maps


def _run(in_maps, **kwargs):
    from concourse import bass_utils

    nc = _get_nc()
    return bass_utils.run_bass_kernel_spmd(
        nc, in_maps, core_ids=list(range(N_HEADS)), **kwargs
    )


def kernel(input, cond, cond_weight, cond_bias):
    input = np.asarray(input, np.float32)
    cond = np.asarray(cond, np.float32)
    cond_weight = np.asarray(cond_weight, np.float32)
    cond_bias = np.asarray(cond_bias, np.float32)
    in_maps = _make_in_maps(input, cond, cond_weight, cond_bias)
    res = _run(in_maps)
    return np.stack([r["out"] for r in res.results], axis=0)


# revision 35
# speedup vs baseline: 11.0453x; 11.0453x over previous
"""Trainium2 Bass kernel for nn_MetaMultiLinear.

Math (per head h, sample b):
    w[b, k]   = sum_c cond[b, c] * CW[k, c] + cb[k]        k = o*17 + i  (544)
    out[b, o] = sum_i x1[b, i] * w[b, o*17+i]              x1 = [input, 1] (17)

Sharding: head h -> NeuronCore h (8 heads, 8 cores), full B=32768 per core.

Split i = 0..15 (needs the per-sample multiply) from i = 16 (x1 = 1, so its
contribution cond1 @ CWones^T + bias goes straight into the output
accumulator).

Per-core device algorithm (tiles of 128 samples, processed in pairs; the
group loop is a For_i hardware loop so the static program stays small —
this runtime's per-execution cost scales with static NEFF size):
  1. One DMA per group loads [cond|1|x|pad] for 2*GRP tiles.
  2. Per pair: one PE transpose (plus a 1x1 fence matmul that carries the
     semaphore waits — transpose-mode matmuls only take one sync wait)
     gives cond1^T at partitions 0-32 / 64-96; ScalarE copies PSUM->SBUF.
  3. Per tile (PE, float32r): W-MM  w1[b, o*16+i] = cond1 @ CWk^T  (K=33,
     N=512, one PSUM bank); po-MM  po[b, o] = cond1 @ CWones^T (start=True,
     opens the tile's accumulation group, carries all bias terms).
  4. Per tile (DVE, the floor: one 1x pass, 512 elem/partition): tmp =
     w1 (*) broadcast(x), reading w1 straight from PSUM.
  5. Per tile (PE, float32r): one reduce matmul with identity stationary
     streams tmp i-outer/o-inner; the PSUM out AP is a broadcast view so
     16 passes accumulate onto po[b, o] via has_written.
  6. ScalarE copies po -> SBUF (DMA cannot read PSUM); one output DMA per
     group. Reduce phases run one pair behind produce phases so the PE
     FIFO has W work while the DVE multiply runs.
"""

import sys

import numpy as np

if "/opt/trn_rl_repo" not in sys.path:
    sys.path.insert(0, "/opt/trn_rl_repo")

N_HEADS, IN_F, COND_IN, OUT_F = 8, 16, 32, 32
B = 32768
INP1 = IN_F + 1  # 17
KW = OUT_F * IN_F  # 512 (i<16 part)
C1 = COND_IN + 1  # 33
P = 128
GRP = 16  # pairs per group

_cached_nc = None

# "overlap": PE grouped reduce via overlapping PSUM out-AP (1 matmul/tile)
# "mm16":    PE grouped reduce via 16 accumulated strided matmuls (sim-safe)
REDUCE_MODE = "overlap"
# float32r: single-pass fast fp32 on PE (1 cycle/row at N>=256; exact fp32
# costs 4 cycles/row). Operands must be typed f32r at their producers.
USE_F32R = True
# use a For_i hardware loop over groups (small static program)
USE_LOOP = True


def _build_nc(b_total=B, grp=None, reps=1, loop=None):
    import concourse.bass as bass
    import concourse.mybir as mybir
    import concourse.tile as tile
    from concourse import bacc
    from contextlib import ExitStack

    f32 = mybir.dt.float32
    fr = mybir.dt.float32r if USE_F32R else f32
    if loop is None:
        loop = USE_LOOP
    nc = bacc.Bacc()
    pairs = b_total // (2 * P)
    if grp is None:
        grp = GRP
    while pairs % grp:
        grp //= 2
    groups = pairs // grp
    gsz = 2 * grp * P  # samples per group

    # cx: per sample [cond (32) | 1.0 | input (16) | zeros (15)]
    cx_t = nc.dram_tensor("cx", [b_total, 64], f32, kind="ExternalInput")
    # cwk[c, o*16+i] = CW[o*17+i, c] (i<16); row 32 = cond_bias slice
    cwk_t = nc.dram_tensor("cwk", [P, KW], fr, kind="ExternalInput")
    # cwo[c, o] = CW[o*17+16, c]; row 32 = cond_bias[o*17+16]
    cwo_t = nc.dram_tensor("cwo", [P, OUT_F], fr, kind="ExternalInput")
    ident_t = nc.dram_tensor("ident", [P, P], fr, kind="ExternalInput")
    out_t = nc.dram_tensor("out", [b_total, OUT_F], f32, kind="ExternalOutput")

    with tile.TileContext(nc) as tc, ExitStack() as ctx:
        consts = ctx.enter_context(tc.tile_pool(name="consts", bufs=1))
        ptrin = ctx.enter_context(tc.tile_pool(name="ptrin", bufs=2))
        ptrs = ctx.enter_context(tc.tile_pool(name="ptrs", bufs=4))
        ptmp = ctx.enter_context(tc.tile_pool(name="ptmp", bufs=4))
        pouts = ctx.enter_context(tc.tile_pool(name="pouts", bufs=2))
        pps_tr = ctx.enter_context(tc.tile_pool(name="pps_tr", bufs=1, space="PSUM"))
        pps_w = ctx.enter_context(tc.tile_pool(name="pps_w", bufs=3, space="PSUM"))
        pps_o = ctx.enter_context(tc.tile_pool(name="pps_o", bufs=2, space="PSUM"))

        cwk = consts.tile([P, KW], fr)
        nc.sync.dma_start(out=cwk, in_=cwk_t[:])
        cwo = consts.tile([P, OUT_F], fr)
        nc.sync.dma_start(out=cwo, in_=cwo_t[:])
        idn = consts.tile([P, P], fr)
        nc.sync.dma_start(out=idn, in_=ident_t[:])
        idn32 = idn.bitcast(f32)

        def emit_group(gb0):
            """Emit one group's program. gb0: starting sample (int or reg)."""
            trin_g = ptrin.tile([P, 2 * grp, 64], f32)
            nc.sync.dma_start(
                out=trin_g[:],
                in_=cx_t[bass.ds(gb0, gsz), :].rearrange(
                    "(t p) c -> p t c", t=2 * grp
                ),
            )
            outs_g = pouts.tile([P, 2 * grp, OUT_F], f32)

            pending = []  # (po, tmps, col)

            def emit_reduce(item):
                po, tmps, col = item
                for t in (0, 1):
                    tmp = tmps[t]
                    if REDUCE_MODE == "overlap":
                        # 16 streamed passes of 32 o-columns accumulate onto
                        # the same PSUM addresses via has_written. i-outer/
                        # o-inner keeps the dst innermost step-1/even/8B-
                        # aligned (fp32r paired PSUM write requirement).
                        rhs = tmp[:].rearrange("p o i -> p i o")
                        ov = (
                            po[:, t, 0:OUT_F]
                            .unsqueeze(1)
                            .broadcast_to([P, IN_F, OUT_F])
                        )
                        nc.tensor.matmul(
                            ov,
                            idn[:],
                            rhs,
                            start=False,
                            stop=True,
                            skip_group_check=True,
                        )
                    else:
                        tv = tmp[:].rearrange("p o i -> p i o")
                        for i in range(IN_F):
                            nc.tensor.matmul(
                                po[:, t, 0:OUT_F],
                                idn[:],
                                tv[:, i, :],
                                start=False,
                                stop=(i == IN_F - 1),
                                skip_group_check=True,
                            )
                # PSUM -> SBUF (DMA cannot read PSUM)
                nc.scalar.copy(out=outs_g[:, col : col + 2, :], in_=po[:, :, 0:OUT_F])

            for pr in range(grp):
                trin = trin_g[:, 2 * pr : 2 * pr + 2, :].rearrange("p t c -> p (t c)")
                trps = pps_tr.tile([P, P], f32)
                # Fence: carries the semaphore waits (trin DMA, idn DMA,
                # trps slot release); transpose-mode matmuls only support a
                # single sync-wait in codegen.
                nc.tensor.matmul(
                    trps[0:1, 0:1],
                    trin[:, 0:1],
                    idn32[:, 0:1],
                    start=True,
                    stop=True,
                    skip_group_check=True,
                )
                nc.tensor.transpose(trps[:], trin[:], idn32[:])
                trs = ptrs.tile([P, P], fr)
                nc.scalar.copy(out=trs[:], in_=trps[:])

                po = pps_o.tile([P, 2, 512], f32)
                tmps = []
                for t in (0, 1):
                    g = t * 64
                    cts = trs[g : g + C1, :]
                    w1 = pps_w.tile([P, KW], f32)
                    nc.tensor.matmul(
                        w1[:],
                        cts,
                        cwk[g : g + C1, :],
                        start=True,
                        stop=True,
                        tile_position=(g, 0),
                    )
                    # opens tile t's accumulation group (own PSUM bank)
                    nc.tensor.matmul(
                        po[:, t, 0:OUT_F],
                        cts,
                        cwo[g : g + C1, :],
                        start=True,
                        stop=False,
                        skip_group_check=True,
                        tile_position=(g, 0),
                    )
                    tmp = ptmp.tile([P, OUT_F, IN_F], fr)
                    w1v = w1[:].rearrange("p (o i) -> p o i", i=IN_F)
                    xv = (
                        trin[:, g + C1 : g + C1 + IN_F]
                        .unsqueeze(1)
                        .broadcast_to([P, OUT_F, IN_F])
                    )
                    nc.vector.tensor_mul(tmp[:], w1v, xv)
                    tmps.append(tmp)
                pending.append((po, tmps, 2 * pr))
                if len(pending) > 1:
                    emit_reduce(pending.pop(0))
            while pending:
                emit_reduce(pending.pop(0))
            nc.sync.dma_start(
                out=out_t[bass.ds(gb0, gsz), :].rearrange(
                    "(t p) o -> p t o", t=2 * grp
                ),
                in_=outs_g[:],
            )

        if loop and groups > 1:
            if reps == 1:
                with tc.For_i(0, groups * gsz, gsz) as iv:
                    emit_group(iv)
            else:
                with tc.For_i(0, reps, 1):
                    with tc.For_i(0, groups * gsz, gsz) as iv:
                        emit_group(iv)
        else:
            for it in range(groups * reps):
                emit_group((it % groups) * gsz)

    nc.compile()
    return nc


def _get_nc():
    global _cached_nc
    if _cached_nc is None:
        _cached_nc = _build_nc()
    return _cached_nc


def _make_in_maps(input, cond, cond_weight, cond_bias):
    ident = np.eye(P, dtype=np.float32)
    in_maps = []
    n_heads, b_total = input.shape[0], input.shape[1]
    for h in range(n_heads):
        cx = np.zeros((b_total, 64), np.float32)
        cx[:, :COND_IN] = cond[h]
        cx[:, COND_IN] = 1.0
        cx[:, C1 : C1 + IN_F] = input[h]
        cw3 = cond_weight[h].reshape(OUT_F, INP1, COND_IN)  # (o, i, c)
        cb2 = cond_bias[h].reshape(OUT_F, INP1)  # (o, i)
        cwk = np.zeros((P, KW), np.float32)
        cwk1 = cw3[:, :IN_F, :].transpose(2, 0, 1).reshape(COND_IN, KW)
        cwk[0:COND_IN] = cwk1
        cwk[COND_IN] = cb2[:, :IN_F].reshape(KW)
        cwk[64 : 64 + COND_IN] = cwk1
        cwk[64 + COND_IN] = cb2[:, :IN_F].reshape(KW)
        cwo = np.zeros((P, OUT_F), np.float32)
        cwo[0:COND_IN] = cw3[:, IN_F, :].T  # [c, o]
        cwo[COND_IN] = cb2[:, IN_F]
        cwo[64 : 64 + COND_IN] = cw3[:, IN_F, :].T
        cwo[64 + COND_IN] = cb2[:, IN_F]
        in_maps.append({"cx": cx, "cwk": cwk, "cwo": cwo, "ident": ident})
    return in_maps


def _run(in_maps, **kwargs):
    from concourse import bass_utils

    nc = _get_nc()
    return bass_utils.run_bass_kernel_spmd(
        nc, in_maps, core_ids=list(range(N_HEADS)), **kwargs
    )


def kernel(input, cond, cond_weight, cond_bias):
    input = np.asarray(input, np.float32)
    cond = np.asarray(cond, np.float32)
    cond_weight = np.asarray(cond_weight, np.float32)
    cond_bias = np.asarray(cond_bias, np.float32)
    in_maps = _make_in_maps(input, cond, cond_weight, cond_bias)
    res = _run(in_maps)
    return np.stack([r["out"] for r in res.results], axis=0)
